# revision 12
# baseline (speedup 1.0000x reference)
"""Trainium2 Bass kernel for nn_AttentionBlock_32238024524154.

A 2-layer cross-attention transformer block (ref returns (c, q)):
    q  = LN(query)
    c  = MHA1(q, LN(context))            # no residual
    c  = c + MLP1(LN(c))
    c  = c + MHA2(q, LN(c))
    c  = c + MLP2(LN(c))

Sharding: data-parallel over batch - 8 batch elements -> 8 NeuronCores,
one element per core, no collectives.

Design (v2 - rebalanced for measured HW rates: PE ~1.8x the cost model,
DVE ~1.35x, ACT ~1.2x, GPSIMD ~0.25x):
  - Residual stream is TOKEN-major [128 tok, 768] in SBUF.  LayerNorm
    stats (bn_stats on DVE) and the normalize-apply (DVE tensor_scalar,
    per-partition rstd/-mu*rstd) run directly on it - no PE transposes
    on the LN input path.  Only the xhat -> feature-major step for
    matmul consumers uses PE transposes (6 per 128-token tile).
  - ln_w/ln_b folded into consumer weights ON THE HOST (exact); k-bias
    dropped (softmax shift invariance); v-bias folded into the
    output-projection bias (fold_bp); both exact.
  - Attention: scores S^T = [keys, queries] as before (two heads of a
    chunk at PE base-partition 0/64), one Exp per [128,1024] PSUM tile.
    AV is FLIPPED: probs pT are the STATIONARY operand [128 keys, 128
    queries] and V [128 keys, 65] (64 dims + ones col) is the MOVING
    operand, so output partitions are 128 queries (vs 65) - halves the
    AV row count on the PE.  Accumulators pack 8 per 2 PSUM banks.
    Normalize = per-partition (per-query) reciprocal * tensor_scalar on
    DVE - no gpsimd partition_broadcast (GPSIMD is 4x slower than the
    model on HW and was on the critical path).
  - Output projection and MLP fc2 emit TOKEN-major [128 tok, 768]
    PSUM; residual enters via an identity-matmul PSUM preload; drains
    are single DVE tensor_tensor adds of the bias broadcast (or
    residual) writing the next residual tile.  Final c4 is written
    fp32 token-major and DMA'd straight out - no output transposes.
  - MLP: hb-streamed weights (3 blocks of 1024 hidden), fc1 feeds Gelu
    on ACT into hT, fc2 accumulates token-major with SBUF bf16 accs.
  - Emission order feeds the PE from the first microsecond: context LN
    + K/V projection first, query LN + Q projection second; layer-2 Q
    projection deferred past attn1 (fills the mlp1 ramp).
"""

import numpy as np
from contextlib import ExitStack

try:
    import concourse.bass as bass
except ImportError:  # pragma: no cover
    import sys

    sys.path.insert(0, "/opt/trn_rl_repo")
    import concourse.bass as bass

import ml_dtypes
import concourse.bacc as bacc
import concourse.tile as tile
from concourse import mybir
from concourse.bass_utils import run_bass_kernel_spmd
from concourse.masks import make_identity

F32 = mybir.dt.float32
BF16 = mybir.dt.bfloat16
AF = mybir.ActivationFunctionType
ALU = mybir.AluOpType

P = 128
D = 768            # model dim
FC = D // P        # 6 feature chunks
DIM = 384          # attention inner dim
QC = DIM // P      # 3 chunks of q/k features
H = 6              # heads
DH = 64            # head dim
NQ = 1024          # query tokens per batch element
NQT = NQ // P      # 8
NK1 = 2048         # context tokens
HID = 3072
HB = 1024          # mlp hidden block
NHB = HID // HB    # 3
HM = HB // P       # 8
EPS = 1e-5
SCALE = DH ** -0.5
N_CORES = 8


def _emit(nc, tc, ctx, io):
    MUL, ADD = ALU.mult, ALU.add

    # ---------------- constants ----------------
    consts = ctx.enter_context(tc.tile_pool(name="consts", bufs=1))

    ident = consts.tile([P, P], BF16)
    make_identity(nc, ident[:])

    epsb = consts.tile([P, 1], F32)
    nc.vector.memset(epsb[:], EPS)
    warmp = ctx.enter_context(tc.tile_pool(name="warm", bufs=2))

    def warm_table(func):
        w = warmp.tile([1, 1], F32, tag="warm")
        nc.scalar.activation(w[:], epsb[0:1, :], func)

    warm_table(AF.Sqrt)

    def feat_major_vec(name, n, eng=None):
        t = consts.tile([P, n // P], F32, tag=f"fmv_{name}")
        (eng or nc.sync).dma_start(t[:], io[name].rearrange("(c p) -> p c", p=P))
        return t

    def bc_vec(name, eng=None):
        # [128, 768] broadcast of a length-768 bf16 vector (stride-0 partition)
        t = consts.tile([P, D], BF16, tag=f"bc_{name}")
        (eng or nc.scalar).dma_start(
            t[:], bass.AP(tensor=io[name].tensor, offset=0, ap=[[0, P], [1, D]]))
        return t

    # ---------------- shared working pools ----------------
    xtok = ctx.enter_context(tc.tile_pool(name="xtok", bufs=6))
    xhatp = ctx.enter_context(tc.tile_pool(name="xhatp", bufs=8))
    stats = ctx.enter_context(tc.tile_pool(name="stats", bufs=4))

    def ln_run(srcs, dstT, psum, dst_col_off=0, q_out=None, dma_eng=None):
        """LayerNorm G token-major [128, D] tiles -> feature-major bf16 dstT.

        srcs: list of ("dram", ap, trow) or ("sb", tile_ap).  Tile i lands at
        dstT cols dst_col_off + i*P.  q_out: (qo_pool, out_ap) to also emit
        the fp32 LN*w+b (query path).  psum: (pool512, pool256)."""
        ps512, ps256 = psum
        G = len(srcs)
        assert G <= 8
        mvb = stats.tile([P, 8, 2], F32, tag="mvb")
        xs = []
        for i, s in enumerate(srcs):
            if s[0] == "dram":
                x = xtok.tile([P, D], F32, tag="xtok")
                eng = (dma_eng[i] if isinstance(dma_eng, list)
                       else (dma_eng or nc.sync))
                eng.dma_start(x[:], s[1][s[2] * P:(s[2] + 1) * P, :])
            else:
                x = s[1]
            xs.append(x)
            st = stats.tile([P, 2, 6], F32, tag="bnst")
            nc.vector.bn_stats(st[:, 0, :], x[:, 0:512])
            nc.vector.bn_stats(st[:, 1, :], x[:, 512:D])
            nc.vector.bn_aggr(mvb[:, i, :], st[:])
        sd = stats.tile([P, 8], F32, tag="sd")
        nc.scalar.activation(sd[:, :G], mvb[:, 0:G, 1], AF.Sqrt, bias=epsb[:])
        rstd = stats.tile([P, 8], F32, tag="rstd")
        nc.vector.reciprocal_approx_fast(rstd[:, :G], sd[:, :G])
        nmr = stats.tile([P, 8], F32, tag="nmr")
        nc.vector.scalar_tensor_tensor(nmr[:, :G], mvb[:, 0:G, 0], -1.0,
                                       rstd[:, :G], op0=MUL, op1=MUL)
        for i in range(G):
            xh = xhatp.tile([P, D], BF16, tag="xhat")
            nc.vector.tensor_scalar(xh[:], xs[i][:], rstd[:, i:i + 1],
                                    nmr[:, i:i + 1], op0=MUL, op1=ADD)
            if q_out is not None:
                qo_pool, q_ap, trow = q_out[0], q_out[1], q_out[2] + i
                qo = qo_pool.tile([P, D], F32, tag="qo")
                nc.vector.tensor_mul(qo[:], xh[:], wbc[:])
                nc.vector.tensor_add(qo[:], qo[:], bbc[:])
                nc.sync.dma_start(q_ap[trow * P:(trow + 1) * P, :], qo[:])
            col = dst_col_off + i * P
            o0 = ps512.tile([P, 512], BF16, tag="lnp")
            for c in range(4):
                nc.tensor.transpose(o0[:, c * P:(c + 1) * P],
                                    xh[:, c * P:(c + 1) * P], ident[:])
            o1 = ps256.tile([P, 256], BF16, tag="lnp1")
            for c in range(2):
                nc.tensor.transpose(o1[:, c * P:(c + 1) * P],
                                    xh[:, (4 + c) * P:(5 + c) * P], ident[:])
            if i % 2 == 0:
                nc.vector.tensor_copy(dstT[:, 0:4, col:col + P],
                                      o0[:].rearrange("p (c t) -> p c t", c=4))
                nc.scalar.copy(dstT[:, 4:6, col:col + P],
                               o1[:].rearrange("p (c t) -> p c t", c=2))
            else:
                nc.scalar.copy(dstT[:, 0:4, col:col + P],
                               o0[:].rearrange("p (c t) -> p c t", c=4))
                nc.vector.tensor_copy(dstT[:, 4:6, col:col + P],
                                      o1[:].rearrange("p (c t) -> p c t", c=2))

    # ---------------- persistent activation pools ----------------
    qTp_pool = tc.alloc_tile_pool(name="qTp", bufs=1, side="right")
    qTp2_pool = ctx.enter_context(tc.tile_pool(name="qTp2", bufs=1))
    cres = ctx.enter_context(tc.tile_pool(name="cres", bufs=16))
    lnct_pool = ctx.enter_context(tc.tile_pool(name="lnct", bufs=2))
    pT_pool = ctx.enter_context(tc.tile_pool(name="pT", bufs=4))
    small = ctx.enter_context(tc.tile_pool(name="small", bufs=4))
    onrm_pool = ctx.enter_context(tc.tile_pool(name="onrm", bufs=4))

    # ---------------- K/V projection for one 512-key block ----------------
    def kv_v_unit(cnap, c0, kb, k4, wkv, v, pskv, act_copies=True):
        ps = pskv.tile([P, 512], F32, tag="pskv")
        for f in range(FC):
            nc.tensor.matmul(
                ps[:, 0:DIM],
                cnap[:, f, c0 + k4 * P:c0 + (k4 + 1) * P],
                wkv[:, f, DIM:2 * DIM],
                start=(f == 0), stop=(f == FC - 1))
        if act_copies:
            nc.scalar.copy(v[:, kb * 4 + k4, :, 0:DH],
                           ps[:, 0:DIM].rearrange("p (h d) -> p h d", h=H))
        else:
            nc.vector.tensor_copy(v[:, kb * 4 + k4, :, 0:DH],
                                  ps[:, 0:DIM].rearrange("p (h d) -> p h d", h=H))

    def kv_k_unit(cnap, c0, kb, qc, wkv, kT, pskv, act_copies=True):
        ps = pskv.tile([P, 512], F32, tag="pskv")
        for f in range(FC):
            nc.tensor.matmul(
                ps[:], wkv[:, f, qc * P:(qc + 1) * P],
                cnap[:, f, c0:c0 + 512],
                start=(f == 0), stop=(f == FC - 1))
        if act_copies:
            nc.scalar.copy(kT[:, qc, kb * 512:(kb + 1) * 512], ps[:])
        else:
            nc.vector.tensor_copy(kT[:, qc, kb * 512:(kb + 1) * 512], ps[:])

    def kv_block(cnap, c0, kb, wkv, kT, v, pskv, act_copies=True):
        for k4 in range(4):
            kv_v_unit(cnap, c0, kb, k4, wkv, v, pskv, act_copies)
        for qc in range(QC):
            kv_k_unit(cnap, c0, kb, qc, wkv, kT, pskv, act_copies)

    # ---------------- attention core + flipped output projection ----------
    def mha_attn(kT, v, m, qTp, wp_name, bp_bc, residual, dst_tiles,
                 on_half=None):
        """Attention + token-major projection.

        kT: [128(hh*64+d), QC, m] bf16; v: [128 keys, m/P, H, DH+1] bf16
        qTp: [128, QC, NQ] bf16 (scaled).  dst_tiles: list of 8 [P, D]
        token-major output tiles (bf16).  residual: list of 8 [P, D] tiles
        or None.  bp_bc: [128, D] broadcast bias tile."""
        mt = m // P
        with tc.tile_pool(name="oT", bufs=1) as oT_pool:
            oT = oT_pool.tile([P, QC, NQ], BF16, tag="oT")
            with tc.tile_pool(name="psS", bufs=2, space="PSUM") as psS_pool, \
                 tc.tile_pool(name="psAcc", bufs=1, space="PSUM") as psAcc, \
                 tc.tile_pool(name="psTr", bufs=2, space="PSUM") as psTr:
                for qh in range(2):
                    for kc in range(QC):
                        accs = [psAcc.tile([P, 4, DH + 1], F32, tag=f"acc{j}",
                                           name=f"acc_{qh}_{kc}_{j}")
                                for j in range(2)]
                        pTs = {}

                        def do_S(kt):
                            psS = psS_pool.tile([P, 1024], F32, tag="psS")
                            for hh in range(2):
                                nc.tensor.matmul(
                                    psS[:, hh * 512:(hh + 1) * 512],
                                    kT[hh * DH:(hh + 1) * DH, kc, kt * P:(kt + 1) * P],
                                    qTp[hh * DH:(hh + 1) * DH, kc, qh * 512:(qh + 1) * 512],
                                    start=True, stop=True)
                            pT = pT_pool.tile([P, 1024], BF16, tag="pT")
                            nc.scalar.activation(pT[:], psS[:], AF.Exp)
                            pTs[kt] = pT

                        def do_av(kt):
                            # start=True zeroes the whole 2KB psum bank, so
                            # only idx 0 of each packed-acc bank starts the
                            # group; idx 3 at the last kt stops it.
                            pT = pTs.pop(kt)
                            for q4 in range(4):
                                acc = accs[q4 // 2]
                                for hh in range(2):
                                    idx = (q4 % 2) * 2 + hh
                                    nc.tensor.matmul(
                                        acc[:, idx, :],
                                        pT[:, hh * 512 + q4 * P:hh * 512 + (q4 + 1) * P],
                                        v[:, kt, 2 * kc + hh, :],
                                        start=(kt == 0 and idx == 0),
                                        stop=(kt == mt - 1 and idx == 3))

                        for kt in range(mt):
                            do_S(kt)
                            if kt >= 2:
                                do_av(kt - 2)
                        do_av(mt - 2)
                        do_av(mt - 1)

                        # normalize by rowsum (col DH), per-query partitions
                        for j in range(2):
                            rcp = small.tile([P, 4], F32, tag="rcp")
                            rs = small.tile([P, 4], F32, tag="rs")
                            nc.vector.tensor_copy(rs[:], accs[j][:, :, DH])
                            nc.vector.reciprocal_approx_fast(rcp[:], rs[:])
                            for qq in range(2):
                                q4 = j * 2 + qq
                                onrm = onrm_pool.tile([P, P], BF16, tag="onrm")
                                for hh in range(2):
                                    nc.vector.tensor_scalar(
                                        onrm[:, hh * DH:(hh + 1) * DH],
                                        accs[j][:, qq * 2 + hh, 0:DH],
                                        rcp[:, qq * 2 + hh:qq * 2 + hh + 1],
                                        None, op0=MUL)
                                # transpose to feature-major oT
                                tr = psTr.tile([P, P], BF16, tag="otr")
                                nc.tensor.transpose(tr[:], onrm[:], ident[:])
                                if q4 % 2 == 0:
                                    nc.scalar.copy(
                                        oT[:, kc, qh * 512 + q4 * P:qh * 512 + (q4 + 1) * P],
                                        tr[:])
                                else:
                                    nc.vector.tensor_copy(
                                        oT[:, kc, qh * 512 + q4 * P:qh * 512 + (q4 + 1) * P],
                                        tr[:])

            # flipped projection: token-major [128 tok, 768] psum
            with tc.tile_pool(name="wp", bufs=1) as wp_pool, \
                 tc.tile_pool(name="psC", bufs=4, space="PSUM") as psC, \
                 tc.tile_pool(name="lnpa", bufs=2, space="PSUM") as lnpa, \
                 tc.tile_pool(name="lnpa1", bufs=2, space="PSUM") as lnpa1:
                wp = wp_pool.tile([P, QC, D], BF16, tag="wp")
                nc.sync.dma_start(wp[:], io[wp_name].rearrange("(c p) d -> p c d", p=P))
                for qh in range(2):
                    for t4 in range(4):
                        t = qh * 4 + t4
                        css = [psC.tile([P, 384], F32, tag="psC",
                                        name=f"psC_{t}_{h}") for h in range(2)]
                        for h in range(2):
                            if residual is not None:
                                nc.tensor.matmul(
                                    css[h][:], ident[:],
                                    residual[t][:, h * 384:(h + 1) * 384],
                                    start=True, stop=False)
                            for qc in range(QC):
                                nc.tensor.matmul(
                                    css[h][:], oT[:, qc, t * P:(t + 1) * P],
                                    wp[:, qc, h * 384:(h + 1) * 384],
                                    start=(residual is None and qc == 0),
                                    stop=(qc == QC - 1))
                        for h in range(2):
                            nc.vector.tensor_add(
                                dst_tiles[t][:, h * 384:(h + 1) * 384],
                                css[h][:], bp_bc[:, h * 384:(h + 1) * 384])
                    if on_half is not None:
                        on_half(qh, (lnpa, lnpa1))

    # ---------------- MLP (token-major fc2) ----------------
    def mlp(lnsrc, w1_name, b1f, w2_name, b2_bc, residual, dst_tiles,
            dst_f32=False, on_half=None):
        with tc.tile_pool(name="w1", bufs=2) as w1_pool, \
             tc.tile_pool(name="w2", bufs=2) as w2_pool, \
             tc.tile_pool(name="hT", bufs=2) as h_pool, \
             tc.tile_pool(name="acc", bufs=8) as acc_pool, \
             tc.tile_pool(name="psf1", bufs=2, space="PSUM") as psf1, \
             tc.tile_pool(name="psf2", bufs=2, space="PSUM") as psf2, \
             tc.tile_pool(name="lnpm", bufs=2, space="PSUM") as lnpm, \
             tc.tile_pool(name="lnpm1", bufs=2, space="PSUM") as lnpm1:
            accs = [acc_pool.tile([P, D], BF16, tag="acc", name=f"acc{t}")
                    for t in range(NQT)]
            w1re = io[w1_name].rearrange("(c p) h -> p c h", p=P)
            w2re = io[w2_name].rearrange("(b p) d -> p b d", p=P)
            for hb in range(NHB):
                w1b = w1_pool.tile([P, FC, HB], BF16, tag="w1b")
                nc.sync.dma_start(w1b[:], w1re[:, :, hb * HB:(hb + 1) * HB])
                w2b = w2_pool.tile([P, HM, D], BF16, tag="w2b")
                nc.sync.dma_start(w2b[:], w2re[:, hb * HM:(hb + 1) * HM, :])
                for qh in range(2):
                    hT = h_pool.tile([P, HM, 512], BF16, tag="hT")
                    for hm in range(HM):
                        hk = hb * HM + hm
                        ps = psf1.tile([P, 512], F32, tag="psf1")
                        for f in range(FC):
                            nc.tensor.matmul(
                                ps[:], w1b[:, f, hm * P:(hm + 1) * P],
                                lnsrc[:, f, qh * 512:(qh + 1) * 512],
                                start=(f == 0), stop=(f == FC - 1))
                        nc.scalar.activation(hT[:, hm, :], ps[:], AF.Gelu,
                                             bias=b1f[:, hk:hk + 1])
                    for t4 in range(4):
                        t = qh * 4 + t4
                        pss = [psf2.tile([P, 384], F32, tag="psf2",
                                         name=f"psf2_{t}_{h}") for h in range(2)]
                        for h in range(2):
                            preload = hb == 0 and residual is not None
                            if preload:
                                nc.tensor.matmul(
                                    pss[h][:], ident[:],
                                    residual[t][:, h * 384:(h + 1) * 384],
                                    start=True, stop=False)
                            for hm in range(HM):
                                nc.tensor.matmul(
                                    pss[h][:], hT[:, hm, t4 * P:(t4 + 1) * P],
                                    w2b[:, hm, h * 384:(h + 1) * 384],
                                    start=(not preload and hm == 0),
                                    stop=(hm == HM - 1))
                        for h in range(2):
                            sl = slice(h * 384, (h + 1) * 384)
                            if hb == 0:
                                nc.vector.tensor_add(accs[t][:, sl], pss[h][:],
                                                     b2_bc[:, sl])
                            elif hb < NHB - 1:
                                nc.vector.tensor_add(accs[t][:, sl],
                                                     accs[t][:, sl], pss[h][:])
                            else:
                                nc.vector.tensor_add(dst_tiles[t][:, sl],
                                                     accs[t][:, sl], pss[h][:])
                    if hb == NHB - 1 and on_half is not None:
                        on_half(qh, (lnpm, lnpm1))

    # ======================= block body =======================
    # ---- phase A: LN(context) + all four K/V projection blocks
    kv1_es = ExitStack()
    kv1_pool = kv1_es.enter_context(tc.tile_pool(name="kv1", bufs=1))
    kT1 = kv1_pool.tile([P, QC, NK1], BF16, tag="kT")
    v1 = kv1_pool.tile([P, NK1 // P, H, DH + 1], BF16, tag="v")
    nc.gpsimd.memset(v1[:, :, :, DH:DH + 1], 1.0)
    wkv1 = kv1_pool.tile([P, FC, 2 * DIM], BF16, tag="wkv")
    nc.gpsimd.dma_start(wkv1[:], io["a1_wkv"].rearrange("(c p) d -> p c d", p=P))
    pstA_es = ExitStack()
    pstA = pstA_es.enter_context(tc.tile_pool(name="pstA", bufs=2, space="PSUM"))
    pstA1 = pstA_es.enter_context(tc.tile_pool(name="pstA1", bufs=2, space="PSUM"))
    with tc.tile_pool(name="cn", bufs=2) as cn_pool, \
         tc.tile_pool(name="pskv", bufs=2, space="PSUM") as pskv:
        for kb in range(4):
            cn = cn_pool.tile([P, FC, 512], BF16, tag="cn")
            if kb == 0:
                ln_run([("dram", io["context"], t) for t in (0, 1)],
                       cn, (pstA, pstA1))
                ln_run([("dram", io["context"], t) for t in (2, 3)],
                       cn, (pstA, pstA1), dst_col_off=256)
            else:
                ln_run([("dram", io["context"], t)
                        for t in range(kb * 4, kb * 4 + 4)], cn, (pstA, pstA1))
            kv_block(cn, 0, kb, wkv1, kT1, v1, pskv)

    # const bias vectors (emitted after the context x-tile DMAs)
    bq1 = feat_major_vec("fold_bq1", DIM)     # (ln_b @ a1_wq) * SCALE
    bq2 = feat_major_vec("fold_bq2", DIM)
    bp1b = bc_vec("fold_bp1")                 # a1_bp + (ln_b @ a1_wv) @ a1_wp
    bp2b = bc_vec("fold_bp2")
    b1m1 = feat_major_vec("m1_b1", HID)       # host-folded (+ ln_b @ m1_w1)
    b2m1b = bc_vec("m1_b2")
    b1m2 = feat_major_vec("m2_b1", HID)
    b2m2b = bc_vec("m2_b2")

    # ---- phase B: LN(query) -> q out + qT; Q projection for layer 1 only
    def q_project(wq, bq, pool, psq):
        qTp = pool.tile([P, QC, NQ], BF16, tag="qTp")
        for qc in range(QC):
            for qhh in range(2):
                ps = psq.tile([P, 512], F32, tag="psq")
                for f in range(FC):
                    nc.tensor.matmul(
                        ps[:], wq[:, f, qc * P:(qc + 1) * P],
                        qT[:, f, qhh * 512:(qhh + 1) * 512],
                        start=(f == 0), stop=(f == FC - 1))
                nc.scalar.activation(
                    qTp[:, qc, qhh * 512:(qhh + 1) * 512], ps[:],
                    AF.Identity, bias=bq[:, qc:qc + 1], scale=SCALE)
        return qTp

    qT_es = ExitStack()
    qT_pool = qT_es.enter_context(tc.tile_pool(name="qT", bufs=1))
    with tc.tile_pool(name="qop", bufs=4) as qo_pool, \
         tc.tile_pool(name="psq", bufs=2, space="PSUM") as psq:
        wbc = qT_pool.tile([P, D], F32, tag="wbc")
        nc.scalar.dma_start(wbc[:], bass.AP(tensor=io["ln_w"].tensor, offset=0,
                                            ap=[[0, P], [1, D]]))
        bbc = qT_pool.tile([P, D], F32, tag="bbc")
        nc.scalar.dma_start(bbc[:], bass.AP(tensor=io["ln_b"].tensor, offset=0,
                                            ap=[[0, P], [1, D]]))
        wqs = []
        for li, wq_name in enumerate(("a1_wq", "a2_wq")):
            wq = qT_pool.tile([P, FC, DIM], BF16, tag="wq", bufs=2,
                              name=f"wq{li}")
            eng = nc.gpsimd if li == 0 else nc.scalar
            eng.dma_start(wq[:], io[wq_name].rearrange("(c p) d -> p c d", p=P))
            wqs.append(wq)
        qT = qT_pool.tile([P, FC, NQ], BF16)
        for g in range(2):
            ln_run([("dram", io["query"], t) for t in range(g * 4, g * 4 + 4)],
                   qT, (pstA, pstA1), dst_col_off=g * 512,
                   q_out=(qo_pool, io["out_q"], g * 4))
        qTp1 = q_project(wqs[0], bq1, qTp_pool, psq)

    # ---- phase C: attention 1 + projection -> c1 tiles (+ streamed LN(c1))
    pstA_es.close()   # free LN psum banks before attention claims them

    c1 = [cres.tile([P, D], BF16, tag="c", name=f"c1_{t}") for t in range(NQT)]
    c2 = [cres.tile([P, D], BF16, tag="c", name=f"c2_{t}") for t in range(NQT)]
    lnc1 = lnct_pool.tile([P, FC, NQ], BF16, tag="lnc", name="lnc1")
    lnc2 = lnct_pool.tile([P, FC, NQ], BF16, tag="lnc", name="lnc2")

    def ln_half_into(dstT, src_tiles):
        def cb(qh, psum):
            ln_run([("sb", src_tiles[t]) for t in range(qh * 4, qh * 4 + 4)],
                   dstT, psum, dst_col_off=qh * 512)
        return cb

    mha_attn(kT1, v1, NK1, qTp1, "a1_wp", bp1b, None, c1,
             on_half=ln_half_into(lnc1, c1))

    # deferred layer-2 Q projection (runs during attn1 drain / mlp1 ramp)
    with tc.tile_pool(name="psq2", bufs=2, space="PSUM") as psq2:
        qTp2 = q_project(wqs[1], bq2, qTp2_pool, psq2)
    qT_es.close()
    kv1_es.close()
    qTp_pool.release()

    # ---- MLP1 (+ streamed LN(c2))
    mlp(lnc1, "m1_w1", b1m1, "m1_w2", b2m1b, c1, c2,
        on_half=ln_half_into(lnc2, c2))

    # ---- MHA2 (keys/values projected from the streamed LN(c2))
    kv2_es = ExitStack()
    kv2_pool = kv2_es.enter_context(tc.tile_pool(name="kv2", bufs=1))
    kT2 = kv2_pool.tile([P, QC, NQ], BF16, tag="kT")
    v2 = kv2_pool.tile([P, NQT, H, DH + 1], BF16, tag="v")
    nc.gpsimd.memset(v2[:, :, :, DH:DH + 1], 1.0)
    wkv2 = kv2_pool.tile([P, FC, 2 * DIM], BF16, tag="wkv")
    nc.sync.dma_start(wkv2[:], io["a2_wkv"].rearrange("(c p) d -> p c d", p=P))

    c3 = [cres.tile([P, D], BF16, tag="c", name=f"c3_{t}") for t in range(NQT)]
    lnc3 = lnct_pool.tile([P, FC, NQ], BF16, tag="lnc", name="lnc3")
    with tc.tile_pool(name="pskv2", bufs=2, space="PSUM") as pskv2_:
        for kb in (0, 1):
            kv_block(lnc2, kb * 512, kb, wkv2, kT2, v2, pskv2_,
                     act_copies=False)

    mha_attn(kT2, v2, NQ, qTp2, "a2_wp", bp2b, c2, c3,
             on_half=ln_half_into(lnc3, c3))
    kv2_es.close()

    # ---- MLP2 -> c4 fp32 token-major -> DMA out
    with tc.tile_pool(name="c4p", bufs=4) as c4_pool:
        c4 = [c4_pool.tile([P, D], F32, tag="c4", name=f"c4_{t}")
              for t in range(NQT)]

        def stream_out(qh, psum):
            for t in range(qh * 4, qh * 4 + 4):
                eng = nc.sync if t % 2 == 0 else nc.scalar
                eng.dma_start(io["out_c"][t * P:(t + 1) * P, :], c4[t][:])

        mlp(lnc3, "m2_w1", b1m2, "m2_w2", b2m2b, c3, c4,
            dst_f32=True, on_half=stream_out)


_BF16_INPUTS = {"a1_wq", "a1_wkv", "a1_wp", "m1_w1", "m1_w2",
                "a2_wq", "a2_wkv", "a2_wp", "m2_w1", "m2_w2",
                "fold_bp1", "fold_bp2", "m1_b2", "m2_b2"}

_IN_SHAPES = {
    "query": (NQ, D), "context": (NK1, D),
    "ln_w": (D,), "ln_b": (D,),
    "a1_wq": (D, DIM), "a1_wkv": (D, 2 * DIM), "a1_wp": (DIM, D),
    "m1_w1": (D, HID), "m1_b1": (HID,), "m1_w2": (HID, D), "m1_b2": (D,),
    "a2_wq": (D, DIM), "a2_wkv": (D, 2 * DIM), "a2_wp": (DIM, D),
    "m2_w1": (D, HID), "m2_b1": (HID,), "m2_w2": (HID, D), "m2_b2": (D,),
    "fold_bq1": (DIM,), "fold_bq2": (DIM,),
    "fold_bp1": (D,), "fold_bp2": (D,),
}

_CACHE = {}


def build(replicas=1):
    key = ("nc", replicas)
    if key in _CACHE:
        return _CACHE[key]
    nc = bacc.Bacc("TRN2", target_bir_lowering=False, debug=False,
                   num_devices=N_CORES)
    io = {}
    for name, shape in _IN_SHAPES.items():
        dt = BF16 if name in _BF16_INPUTS else F32
        io[name] = nc.dram_tensor(name, list(shape), dt, kind="ExternalInput").ap()
    io["out_c"] = nc.dram_tensor("out_c", [NQ, D], F32, kind="ExternalOutput").ap()
    io["out_q"] = nc.dram_tensor("out_q", [NQ, D], F32, kind="ExternalOutput").ap()
    with tile.TileContext(nc) as tc:
        for _ in range(replicas):
            with ExitStack() as ctx:
                _emit(nc, tc, ctx, io)
    nc.compile()
    _CACHE[key] = nc
    return nc


def host_fold(inputs):
    """Fold ln_w into consumer weights; fold ln_b-induced bias terms; fold
    the v-bias through the output projection.  All in f64, exact."""
    f = np.float32
    bf = ml_dtypes.bfloat16
    w = np.asarray(inputs["ln_w"], np.float64)
    b = np.asarray(inputs["ln_b"], np.float64)
    out = {}
    for k, v_ in inputs.items():
        if k in _IN_SHAPES:
            out[k] = np.ascontiguousarray(np.asarray(v_, f))
    scaled = {}
    for wn in ("a1_wq", "a1_wkv", "m1_w1", "a2_wq", "a2_wkv", "m2_w1"):
        scaled[wn] = np.asarray(inputs[wn], np.float64) * w[:, None]
        out[wn] = np.ascontiguousarray(scaled[wn].astype(f))
    out["fold_bq1"] = (b @ scaled["a1_wq"] * SCALE).astype(f)
    out["fold_bq2"] = (b @ scaled["a2_wq"] * SCALE).astype(f)
    for li in (1, 2):
        wkv = scaled[f"a{li}_wkv"]
        wp = np.asarray(inputs[f"a{li}_wp"], np.float64)
        bp = np.asarray(inputs[f"a{li}_bp"], np.float64)
        bv = b @ wkv[:, DIM:]
        out[f"fold_bp{li}"] = (bp + bv @ wp).astype(f)
    out["m1_b1"] = (np.asarray(inputs["m1_b1"], np.float64)
                    + b @ scaled["m1_w1"]).astype(f)
    out["m2_b1"] = (np.asarray(inputs["m2_b1"], np.float64)
                    + b @ scaled["m2_w1"]).astype(f)
    for wn in _BF16_INPUTS:
        out[wn] = np.ascontiguousarray(out[wn].astype(bf))
    return out


def kernel(**inputs):
    nc = build()
    folded = host_fold(inputs)
    in_maps = []
    for i in range(N_CORES):
        m = {}
        for name in _IN_SHAPES:
            a = folded[name]
            if name in ("query", "context"):
                a = np.ascontiguousarray(np.asarray(inputs[name], np.float32)[i])
            m[name] = a
        in_maps.append(m)
    res = run_bass_kernel_spmd(nc, in_maps, list(range(N_CORES)))
    c = np.stack([res.results[i]["out_c"] for i in range(N_CORES)])
    q = np.stack([res.results[i]["out_q"] for i in range(N_CORES)])
    return (c, q)
